# revision 29
# baseline (speedup 1.0000x reference)
"""Trainium2 Bass kernel for nn_AdaptiveExpertSystem (MoE, E=8, top-2).

Expert-parallel design: the host computes the (cheap) router on CPU and
uses it as the sharding function — each of the 8 cores receives exactly
the tokens routed to its expert, pre-normalized (the LN the router
already computed) and pre-transposed to feature-major layout. The device
does the FLOPs: the expert MLP (mm1 -> exact gelu -> mm2) in bf16 with
both weight matrices resident in SBUF and tokens streaming through
chunk by chunk, at the PE's bf16 matmul roofline. ln_g/ln_b are folded
into w1/b1 on the host so all experts share the plain LN. The host
scatter-adds each expert's output back with the top-2 combine weights
(the unshard step).

Engine layout notes (from perfetto traces):
- weight DMAs are many small per-tile transfers on the gpsimd queue:
  the gpsimd engine stalls on DMA ring slots, but it has no other
  time-critical work, and small transfers keep the fabric fair so the
  token stream isn't starved behind 16.8 MB of weights.
- sync/scalar queues carry the token chunks (and output), so mm1 can
  start ~10us into the kernel while weights trickle in underneath.
"""
import numpy as np
import ml_dtypes

import concourse.bass as bass
import concourse.tile as tile
from concourse import bacc, mybir
from concourse.bass_utils import run_bass_kernel_spmd

N_CORES = 8
B, L, D, I, E = 2, 2048, 1024, 4096, 8
NTOK = B * L
KD = D // 128       # 8  d-tiles (contraction of mm1)
NI = I // 128       # 32 i-tiles (contraction of mm2)
ND = D // 128       # 8  output d-tiles
WG = 4              # weight DMA groups
LN_EPS = 1e-5

F32 = mybir.dt.float32
BF16 = mybir.dt.bfloat16
BF = ml_dtypes.bfloat16

_CACHE = {}


def _chunks(cap_mm):
    # small first chunk (fast first-token delivery -> earlier mm1 start),
    # remainder split ~evenly; every chunk >= 232 cols keeps LDWEIGHTS
    # hidden under the moving-operand stream
    ws = [256]
    rest = cap_mm - 256
    n = max(1, (rest + 447) // 448)
    base = rest // n
    ws += [base] * n
    ws[-1] += rest - base * n
    out, o = [], 0
    for w in ws:
        out.append((o, w))
        o += w
    return out


def build_nc(cap_mm):
    chunks = _chunks(cap_mm)

    nc = bacc.Bacc(None, num_devices=N_CORES)
    # token chunks, feature-major, one DRAM blob per chunk
    xt_ps = [nc.declare_dram_parameter(f"xt{j}", [128, KD, w], BF16,
                                       isOutput=False)
             for j, (_, w) in enumerate(chunks)]
    w1_p = nc.declare_dram_parameter("w1", [WG, 128, NI // WG, KD, 128], BF16,
                                     isOutput=False)
    w2_p = nc.declare_dram_parameter("w2", [WG, 128, ND // WG, NI, 128], BF16,
                                     isOutput=False)
    b1_p = nc.declare_dram_parameter("b1", [128, NI], F32, isOutput=False)
    out_p = nc.declare_dram_parameter("out", [ND, 128, cap_mm], BF16,
                                      isOutput=True)

    AF = mybir.ActivationFunctionType

    from contextlib import ExitStack
    with tile.TileContext(nc) as tc, ExitStack() as ctx:
        ep = ctx.enter_context
        xntp = ep(tc.tile_pool(name="xnt", bufs=1))
        w1pool = ep(tc.tile_pool(name="w1p", bufs=1))
        w2pool = ep(tc.tile_pool(name="w2p", bufs=1))
        b1pool = ep(tc.tile_pool(name="b1p", bufs=1))
        h1pool = ep(tc.tile_pool(name="h1p", bufs=1))
        h2pool = ep(tc.tile_pool(name="h2p", bufs=2))
        ps1 = ep(tc.tile_pool(name="ps1", bufs=4, space="PSUM"))
        ps2 = ep(tc.tile_pool(name="ps2", bufs=4, space="PSUM"))

        # ALL input DMAs on one queue (gpsimd) in exact consumption order:
        # chunk-0 tokens, w1 tiles, chunk-1, w2 tiles, chunk-2. With all 8
        # cores pulling concurrently HBM is the binding constraint at the
        # head, so global ordering by need-time beats queue parallelism.
        # The gpsimd engine stalls on DMA ring slots but has no other work.
        xnT = xntp.tile([128, KD, cap_mm], BF16)
        b1sb = b1pool.tile([128, NI], F32)
        nc.scalar.dma_start(out=b1sb, in_=b1_p[:])
        ig = NI // WG
        dg = ND // WG
        w1sb = w1pool.tile([128, NI, KD, 128], BF16)
        w2sb = w2pool.tile([128, ND, NI, 128], BF16)

        (c0_0, w_0) = chunks[0]
        for kk in range(0, KD, 2):
            nc.gpsimd.dma_start(out=xnT[:, kk:kk + 2, c0_0:c0_0 + w_0],
                                in_=xt_ps[0][:, kk:kk + 2, :])
        for g in range(WG):
            for i in range(ig):
                nc.gpsimd.dma_start(out=w1sb[:, g * ig + i],
                                    in_=w1_p[g, :, i])
        if len(chunks) > 1:
            (c0_1, w_1) = chunks[1]
            nc.gpsimd.dma_start(out=xnT[:, :, c0_1:c0_1 + w_1],
                                in_=xt_ps[1][:])
        for g in range(WG):
            for d in range(dg):
                nc.gpsimd.dma_start(out=w2sb[:, g * dg + d, 0:NI // 2],
                                    in_=w2_p[g, :, d, 0:NI // 2])
                nc.gpsimd.dma_start(out=w2sb[:, g * dg + d, NI // 2:],
                                    in_=w2_p[g, :, d, NI // 2:])
        for j, (c0, w) in list(enumerate(chunks))[2:]:
            nc.gpsimd.dma_start(out=xnT[:, :, c0:c0 + w], in_=xt_ps[j][:])

        for ci, (c0, w) in enumerate(chunks):
            # ---- mm1 + gelu -> h1 (this chunk) ----
            h1 = h1pool.tile([128, NI, w], BF16, name=f"h1_{ci}", tag="h1",
                             bufs=1)
            for i in range(NI):
                p1 = ps1.tile([128, w], F32, tag="p1", name=f"p1_{ci}_{i}")
                for k in range(KD):
                    nc.tensor.matmul(p1, lhsT=w1sb[:, i, k],
                                     rhs=xnT[:, k, c0:c0 + w],
                                     start=(k == 0), stop=(k == KD - 1))
                nc.scalar.activation(out=h1[:, i], in_=p1, func=AF.Gelu,
                                     bias=b1sb[:, i:i + 1], scale=1.0)
            # ---- mm2 -> h2 -> DMA out ----
            for d in range(ND):
                p2 = ps2.tile([128, w], F32, tag="p2", name=f"p2_{ci}_{d}")
                for i in range(NI):
                    nc.tensor.matmul(p2, lhsT=w2sb[:, d, i], rhs=h1[:, i],
                                     start=(i == 0), stop=(i == NI - 1))
                h2 = h2pool.tile([128, w], BF16, tag="h2",
                                 name=f"h2_{ci}_{d}")
                nc.vector.tensor_copy(out=h2, in_=p2)
                nc.sync.dma_start(out=out_p[d][:, c0:c0 + w], in_=h2)

    nc.finalize()
    return nc


def _pack_w1(w1e):
    # [d, i] -> [g, p, i_sub, k, m]; d = k*128 + p, i = (g*ig + i_sub)*128 + m
    t = w1e.reshape(KD, 128, WG, NI // WG, 128)
    return np.ascontiguousarray(t.transpose(2, 1, 3, 0, 4)).astype(BF)


def _pack_w2(w2e):
    # [i, d] -> [g, p, d_sub, i_tile, m]; i = it*128 + p,
    # d = (g*dg + d_sub)*128 + m
    t = w2e.reshape(NI, 128, WG, ND // WG, 128)
    return np.ascontiguousarray(t.transpose(2, 1, 3, 0, 4)).astype(BF)


def kernel(**inputs) -> np.ndarray:
    x = np.asarray(inputs["hidden_states"], np.float32).reshape(NTOK, D)
    rn_g = np.asarray(inputs["rn_g"], np.float32)
    rn_b = np.asarray(inputs["rn_b"], np.float32)
    router_w = np.asarray(inputs["router_w"], np.float32)
    router_b = np.asarray(inputs["router_b"], np.float32)
    ln_g = np.asarray(inputs["ln_g"], np.float32)
    ln_b = np.asarray(inputs["ln_b"], np.float32)
    w1 = np.asarray(inputs["w1"], np.float32)
    b1 = np.asarray(inputs["b1"], np.float32)
    w2 = np.asarray(inputs["w2"], np.float32)
    b2 = np.asarray(inputs["b2"], np.float32)

    # ---- Router on host: this IS the sharding function ----
    m = x.mean(-1, keepdims=True)
    v = ((x - m) ** 2).mean(-1, keepdims=True)
    rstd = 1.0 / np.sqrt(v + LN_EPS)
    normed = (x - m) * rstd
    logits = (normed * rn_g + rn_b) @ router_w.T + router_b
    top2 = np.argsort(-logits, axis=-1, kind="stable")[:, :2]
    tv = np.take_along_axis(logits, top2, -1)
    tv = np.exp(tv - tv.max(-1, keepdims=True))
    tw = (tv / tv.sum(-1, keepdims=True)).astype(np.float32)

    idxs, wts = [], []
    for e in range(E):
        sel = (top2[:, 0] == e) | (top2[:, 1] == e)
        idx_e = np.nonzero(sel)[0]
        w_e = np.where(top2[idx_e, 0] == e, tw[idx_e, 0], tw[idx_e, 1])
        idxs.append(idx_e)
        wts.append(w_e.astype(np.float32))
    max_n = max(len(ix) for ix in idxs)
    cap_mm = max(256, ((max_n + 7) // 8) * 8)
    chunks = _chunks(cap_mm)

    # ---- Per-core inputs: normalized gathered tokens (feature-major,
    # chunk blobs) + this expert's weights ----
    normed_bf = normed.astype(BF)
    in_maps = []
    for e in range(E):
        g = np.zeros((cap_mm, D), dtype=BF)
        g[:len(idxs[e])] = normed_bf[idxs[e]]
        # [c, d] -> [p, k, c]
        t = g.T.reshape(KD, 128, cap_mm).transpose(1, 0, 2)
        w1e = ln_g[e][:, None] * w1[e]
        b1e = b1[e] + ln_b[e] @ w1[e]
        mp = {
            "w1": _pack_w1(w1e),
            "w2": _pack_w2(w2[e]),
            "b1": np.ascontiguousarray(b1e.reshape(NI, 128).T
                                       .astype(np.float32)),
        }
        for j, (c0, w) in enumerate(chunks):
            mp[f"xt{j}"] = np.ascontiguousarray(t[:, :, c0:c0 + w])
        in_maps.append(mp)

    key = cap_mm
    if key not in _CACHE:
        _CACHE[key] = build_nc(cap_mm)
    nc = _CACHE[key]
    res = run_bass_kernel_spmd(nc, in_maps, core_ids=list(range(N_CORES)))

    # ---- Unshard: weighted scatter-add (top-2 combine) ----
    out = tw[:, 0:1] * b2[top2[:, 0]] + tw[:, 1:2] * b2[top2[:, 1]]
    for e in range(E):
        h2 = np.asarray(res.results[e]["out"], dtype=np.float32)
        h2 = h2.reshape(D, cap_mm).T          # [cap_mm, D]
        out[idxs[e]] += wts[e][:, None] * h2[:len(idxs[e])]
    return out.reshape(B, L, D).astype(np.float32)
